# revision 111
# baseline (speedup 1.0000x reference)
"""MHA kernel for 8 Trainium2 NeuronCores.

Reference computation (per batch b):
    Qh = (q[b] @ Wq.T) * Dh^-0.5, Kh = k[b] @ Wk.T, Vh = v[b] @ Wv.T   (split into 16 heads of 128)
    P  = softmax(Qh Kh^T), O = P Vh, out[b] = concat_heads(O) @ Wo.T
Mask is all-False (spec fill=zeros) and is ignored.

Sharding: 8 cores = 2 batches x 4 head-groups (4 heads / core).
Wq/Wk/Wv are split column-wise (output dims), Wo row-wise (input dims);
the all-reduce after the output projection is done on the host during the
gather (sum of the 4 per-head-group partial projections per batch).

Per-core device kernel (all matmul operands bf16, PSUM accumulation fp32):
  inputs (host-prepared): xq/xk/xv = x[b].T [D,S]; wq/wk/wv = W_slice.T [D,512]
  (Dh^-0.5 folded into wq); wo = Wo_slice.T [512, D].

Schedule: the attention phase is ACT(exp)-throughput-bound, so the V
projection and the output projection are interleaved into it as PE filler
work.  Sq is processed in halves of 1024 so a head's O accumulator needs
only 2 PSUM banks; one PSUM pool with 8 single-bank tags spans all phases
so bank reuse waits only on that bank's last consumer (no pool barriers).
Per Sq-half, region structure (fillers in brackets):
  R1a: scores+exp h0 [fillers]      R1b: scores h1 [fillers]
  R2:  O h0, scores h2 [fillers]    R3: O h1, scores h3 [fillers]
  R4:  O h2 [fillers]               R5: O h3 [fillers]
half0 fillers: V-projection units (vh[m] must land before O(h0,m) in R2);
half1 fillers: out-projection units for rows m<8; rows m>=8 run after,
with kh=0..2 of the first rows pre-staged so the final h3 normalize
latency stays hidden.  Softmax denominators: pair-sum tree on DVE (bf16,
depth 3; depth 1 for the half-final head so its ones-matmuls spread over
the m-steps instead of clustering DVE adds at the half boundary), then a
ones-matmul contraction into ps_d; 1/denom is broadcast across partitions
on the otherwise-idle GpSimd engine.  Weight/x DMA loads are sliced and
interleaved so the first projection matmul starts ~3.5us in; the output is
stored bf16 (the host accumulates the 4 partial projections in f32).
"""

import numpy as np
import ml_dtypes

BF16 = ml_dtypes.bfloat16

B = 2
S = 2048
D = 2048
NH_TOT = 16
DH = 128
H = 4            # heads per core
HS = H * DH      # 512, model-dim slice per core
P = 128
KD = D // P      # 16 contraction tiles over model dim
MT = S // P      # 16 seq tiles
N4 = S // 512    # 4 column groups of 512

_CACHE: dict = {}


def _build_bass():
    import concourse.tile as tile
    from concourse import bacc, mybir

    f32 = mybir.dt.float32
    bf16 = mybir.dt.bfloat16
    Exp = mybir.ActivationFunctionType.Exp

    nc = bacc.Bacc()

    xq = nc.declare_dram_parameter("xq", [D, S], bf16, isOutput=False)
    xk = nc.declare_dram_parameter("xk", [D, S], bf16, isOutput=False)
    xv = nc.declare_dram_parameter("xv", [D, S], bf16, isOutput=False)
    wq = nc.declare_dram_parameter("wq", [D, HS], bf16, isOutput=False)
    wk = nc.declare_dram_parameter("wk", [D, HS], bf16, isOutput=False)
    wv = nc.declare_dram_parameter("wv", [D, HS], bf16, isOutput=False)
    wo = nc.declare_dram_parameter("wo", [HS, D], bf16, isOutput=False)
    out = nc.declare_dram_parameter("out", [S, D], bf16, isOutput=True)

    dma = nc.default_dma_engine

    with tile.TileContext(nc) as tc:
        with (
            tc.sbuf_pool(name="const", bufs=1) as cpool,
            tc.sbuf_pool(name="persist", bufs=1) as ppool,
            tc.sbuf_pool(name="small", bufs=2) as spool,
            tc.sbuf_pool(name="ostage", bufs=6) as opool,
            tc.psum_pool(name="pp", bufs=1) as pp,
        ):
            ones = cpool.tile([P, P], bf16, tag="ones")
            nc.vector.memset(ones, 1.0)

            qhT = ppool.tile([P, H, S], bf16, tag="qhT")   # [Dh, h, Sq]
            khT = ppool.tile([P, H, S], bf16, tag="khT")   # [Dh, h, Sk]
            vh = ppool.tile([P, MT, HS], bf16, tag="vh")   # [seq_p, m, 4*Dh]
            oT = ppool.tile([P, H, S], bf16, tag="oT")     # [Dh, h, Sq] normalized
            wo_sb = ppool.tile([P, H, D], bf16, tag="wo_sb")

            # single-bank psum tags; every allocation waits only on that
            # bank's previous consumer (bufs=1, no pool barriers)
            PS_TAGS = ["ps_s0", "ps_s1", "ps_oA0", "ps_oA1",
                       "ps_oB0", "ps_oB1", "ps_d", "mx"]

            def ps(tag_i, name="psb"):
                return pp.tile([P, 512], f32, tag=PS_TAGS[tag_i], bufs=1,
                               name=name)

            # scores-psum rotation: regions with no PE filler work use a
            # deeper rotation (over banks whose O-accumulators are idle) so
            # PE can run ahead of ACT; elsewhere 2 banks suffice
            sc_rot = {"set": [0, 1], "i": 0}

            def ps_sc():
                sc_rot["i"] = (sc_rot["i"] + 1) % len(sc_rot["set"])
                return ps(sc_rot["set"][sc_rot["i"]], name="ps_sc")

            from contextlib import ExitStack
            stk = ExitStack()
            # xv/wv live in their own (non-reused) address range so their
            # loads can prefetch during the K projection
            wvpool = stk.enter_context(tc.sbuf_pool(name="wvp", bufs=1))
            xvpool = stk.enter_context(tc.sbuf_pool(name="xvp", bufs=16))
            wv_sb = wvpool.tile([P, KD, HS], bf16, tag="wv_sb")
            xvt = []

            # ---------------- QK projections ----------------
            with (
                tc.sbuf_pool(name="wqk", bufs=1) as wpool,
                tc.sbuf_pool(name="xs", bufs=18) as xpool,
            ):
                wq_sb = wpool.tile([P, KD, HS], bf16, tag="wq_sb")
                wk_sb = wpool.tile([P, KD, HS], bf16, tag="wk_sb")

                def load_w(w_sb, w_dram, s):
                    src = w_dram.rearrange("(k p) n -> p k n", p=P)
                    dma.dma_start(
                        w_sb[:, 4 * s:4 * (s + 1), :], src[:, 4 * s:4 * (s + 1), :]
                    )

                def proj_qk(x_dram, w_sb, out_sb, w_dram, first=False):
                    for nh in range(2):  # S halves, 1024 wide
                        xt = []
                        for kd in range(KD):
                            if nh == 0 and kd % 4 == 0:
                                # interleave weight-slice loads with the x
                                # stream so arrival order matches consumption
                                if first and kd == 0:
                                    src = w_dram.rearrange("(k p) n -> p k n", p=P)
                                    dma.dma_start(w_sb[:, 0:1, :], src[:, 0:1, :])
                                else:
                                    load_w(w_sb, w_dram, kd // 4)
                            xti = xpool.tile([P, 1024], bf16, tag="xt")
                            if first and kd == 0:
                                # first kd: one wq slice, then the x halves —
                                # the opening matmul's inputs transfer first
                                dma.dma_start(
                                    xti[:, 0:512], x_dram[0:P, nh * 1024:nh * 1024 + 512]
                                )
                                dma.dma_start(
                                    xti[:, 512:1024],
                                    x_dram[0:P, nh * 1024 + 512:(nh + 1) * 1024],
                                )
                            else:
                                dma.dma_start(
                                    xti,
                                    x_dram[kd * P:(kd + 1) * P, nh * 1024:(nh + 1) * 1024],
                                )
                            if first and nh == 0 and kd < 3:
                                # single-kd weight slices interleaved with the
                                # x stream so neither delays the other early on
                                src = w_dram.rearrange("(k p) n -> p k n", p=P)
                                dma.dma_start(
                                    w_sb[:, kd + 1:kd + 2, :], src[:, kd + 1:kd + 2, :]
                                )
                            xt.append(xti)
                        pss = [ps(t, name="ps_proj") for t in range(8)]
                        for kd in range(KD):
                            for t in range(8):
                                nc.tensor.matmul(
                                    pss[t],
                                    lhsT=w_sb[:, kd, (t // 2) * P:(t // 2 + 1) * P],
                                    rhs=xt[kd][:, (t % 2) * 512:(t % 2 + 1) * 512],
                                    start=(kd == 0),
                                    stop=(kd == KD - 1),
                                )
                        for t in range(8):
                            h, n = t // 2, t % 2
                            dst = out_sb[:, h, nh * 1024 + n * 512: nh * 1024 + (n + 1) * 512]
                            if t % 2 == 0:
                                nc.scalar.copy(dst, pss[t])
                            else:
                                nc.vector.tensor_copy(dst, pss[t])

                proj_qk(xq, wq_sb, qhT, wq, first=True)
                proj_qk(xk, wk_sb, khT, wk)

                # prefetch V inputs + output weights while K projects
                src_wv = wv.rearrange("(k p) n -> p k n", p=P)
                for s in range(4):
                    dma.dma_start(
                        wv_sb[:, 4 * s:4 * (s + 1), :],
                        src_wv[:, 4 * s:4 * (s + 1), :],
                    )
                for nh in range(2):
                    for kd in range(KD):
                        xvi = xvpool.tile([P, 1024], bf16, tag="xv")
                        dma.dma_start(
                            xvi,
                            xv[kd * P:(kd + 1) * P, nh * 1024:(nh + 1) * 1024],
                        )
                        xvt.append(xvi)
                src_wo = wo.rearrange("(k p) n -> p k n", p=P)
                for s in range(4):
                    dma.dma_start(wo_sb[:, s, :], src_wo[:, s, :])

            # pt pool reuses the wqk/xs address range; first writes wait on
            # the last QK-projection readers automatically
            if True:
                ptpool = stk.enter_context(tc.sbuf_pool(name="pts", bufs=34))

                # ---------------- fused attention + V/out projections ------
                pt = {}          # (h) -> list of P^T tiles [P, 1024] current half
                ps_o = {}        # (h) -> [2 psum tiles] current half
                ps_dd = {}       # (h) -> denominator psum tile current half

                def sc_step(h, half, m):
                    pti = ptpool.tile([P, 1024], bf16, tag="pt", name="pt")
                    for n in range(2):
                        s_ps = ps_sc()
                        nc.tensor.matmul(
                            s_ps,
                            lhsT=khT[:, h, m * P:(m + 1) * P],
                            rhs=qhT[:, h, half * 1024 + n * 512:
                                    half * 1024 + (n + 1) * 512],
                        )
                        nc.scalar.activation(
                            pti[:, n * 512:(n + 1) * 512], s_ps, Exp
                        )
                    pt[h].append(pti)

                def o_step(h, m):
                    if m == 0:
                        base = 2 if h % 2 == 0 else 4
                        ps_o[h] = [ps(base + n, name="ps_o") for n in range(2)]
                        ps_dd[h] = ps(6, name="ps_dd")
                    for n in range(2):
                        nc.tensor.matmul(
                            ps_o[h][n],
                            lhsT=vh[:, m, h * P:(h + 1) * P],
                            rhs=pt[h][m][:, n * 512:(n + 1) * 512],
                            start=(m == 0),
                            stop=(m == MT - 1),
                        )
                    # pair-sum tree (DVE, bf16) lagging the O consumption of
                    # the pt tiles, then a ones-matmul contraction into ps_dd.
                    # Sums accumulate into the odd-index tile so the
                    # even-index pool slots free as early as possible.
                    # The half-final head (h3) uses a depth-1 tree: its
                    # ones-matmuls spread over the odd m-steps instead of
                    # clustering DVE adds at the half boundary where the PE
                    # would stall on them.
                    if m % 2 == 1:
                        j = m // 2
                        nc.vector.tensor_add(
                            pt[h][2 * j + 1], pt[h][2 * j + 1], pt[h][2 * j]
                        )
                        if h == 3:
                            for n in range(2):
                                nc.tensor.matmul(
                                    ps_dd[h][32 * n:32 * n + 1, :],
                                    lhsT=ones[:, 0:1],
                                    rhs=pt[h][2 * j + 1][:, n * 512:(n + 1) * 512],
                                    start=(j == 0),
                                    stop=(j == MT // 2 - 1),
                                    tile_position=(0, 32 * n),
                                )
                        elif j % 2 == 1:
                            i = j // 2
                            nc.vector.tensor_add(
                                pt[h][4 * i + 3], pt[h][4 * i + 3], pt[h][4 * i + 1]
                            )
                            if i % 2 == 1:
                                g = i // 2
                                nc.vector.tensor_add(
                                    pt[h][8 * g + 7], pt[h][8 * g + 7],
                                    pt[h][8 * g + 3],
                                )
                                for n in range(2):
                                    nc.tensor.matmul(
                                        ps_dd[h][32 * n:32 * n + 1, :],
                                        lhsT=ones[:, 0:1],
                                        rhs=pt[h][8 * g + 7][:, n * 512:(n + 1) * 512],
                                        start=(g == 0),
                                        stop=(g == 1),
                                        tile_position=(0, 32 * n),
                                    )

                def normalize(h, half):
                    for n in range(2):
                        rb_row = spool.tile([1, 512], bf16, tag="rb_row")
                        # bf16 reciprocal matches the old bf16 d_bf roundtrip
                        with nc.allow_low_precision(reason="softmax denom bf16"):
                            nc.vector.reciprocal(
                                rb_row, ps_dd[h][32 * n:32 * n + 1, :]
                            )
                        rb = spool.tile([P, 512], bf16, tag="rb")
                        nc.gpsimd.partition_broadcast(rb, rb_row)
                        nc.vector.tensor_mul(
                            oT[:, h, half * 1024 + n * 512:
                               half * 1024 + (n + 1) * 512],
                            ps_o[h][n], rb,
                        )
                    pt[h] = []

                def v_unit(m, tag=7):
                    psv = ps(tag, name="ps_v")
                    nh = m // 8
                    for kd in range(KD):
                        nc.tensor.matmul(
                            psv,
                            lhsT=xvt[nh * KD + kd][:, (m % 8) * P:(m % 8 + 1) * P],
                            rhs=wv_sb[:, kd, :],
                            start=(kd == 0),
                            stop=(kd == KD - 1),
                        )
                    nc.scalar.copy(vh[:, m, :], psv)

                def op_unit(m, n, eng, tag=7, split=False):
                    psf = ps(tag, name="ps_f")
                    for kh in range(H):
                        nc.tensor.matmul(
                            psf,
                            lhsT=oT[:, kh, m * P:(m + 1) * P],
                            rhs=wo_sb[:, kh, n * 512:(n + 1) * 512],
                            start=(kh == 0),
                            stop=(kh == H - 1),
                        )
                    ob = opool.tile([P, 512], bf16, tag="ob")
                    if split:
                        # kernel-tail drains: use both engines per tile and
                        # issue the store from an otherwise-idle sequencer so
                        # the final DMAs don't serialize behind SP
                        nc.scalar.copy(ob[:, 0:256], psf[:, 0:256])
                        nc.vector.tensor_copy(ob[:, 256:512], psf[:, 256:512])
                        store_eng = nc.scalar if n % 2 == 0 else dma
                    elif eng == 0:
                        nc.scalar.copy(ob, psf)
                        store_eng = dma
                    else:
                        nc.vector.tensor_copy(ob, psf)
                        store_eng = dma
                    store_eng.dma_start(
                        out[m * P:(m + 1) * P, n * 512:(n + 1) * 512], ob
                    )

                from collections import deque

                # per-region filler slots (by m), sized so cumulative PE work
                # stays ahead of the exps each region's O matmuls depend on
                PLACE = {
                    0: {  # half0: 16 V-projection units; all vh[m] must land
                          # before O(h0, m) consumes them in R2
                        "R1a": {3, 7, 11, 15},
                        "R1b": {1, 3, 5, 7, 9, 11, 13, 15},
                        "R2": {0, 4, 8, 12},
                        "R3": set(), "R4": set(), "R5": set(),
                    },
                    1: {  # half1: 32 out-projection units for rows m<8;
                          # R1a starts at m=4 so the h3 normalize latency of
                          # the previous half hides under plain score steps
                        "R1a": set(range(4, MT)),
                        "R1b": {0, 2, 5, 8, 11, 13, 14, 15},
                        "R2": {0, 2, 5, 8, 11, 14},
                        "R3": {3, 7, 11, 15},
                        "R4": {5, 11},
                        "R5": set(),
                    },
                }

                # filler psum-bank rotation per region: banks whose
                # O-accumulators are idle in that region join the rotation so
                # a unit never waits on the previous unit's drain.  half1's
                # R1a/R1b carry many fillers, so give them banks 4-7 and
                # cap the scores rotation at 0-3 there.
                FTAGS = {
                    0: {"R1a": [6, 7], "R1b": [6, 7], "R2": [7],
                        "R3": [2, 3, 7], "R4": [4, 5, 7], "R5": [2, 3, 7]},
                    1: {"R1a": [4, 5, 6, 7], "R1b": [4, 5, 6, 7], "R2": [7],
                        "R3": [2, 3, 7], "R4": [4, 5, 7], "R5": [2, 3, 7]},
                }

                for half in range(2):
                    for h in range(H):
                        pt[h] = []
                    if half == 0:
                        fillers = deque(
                            lambda m=m, t=0: v_unit(m, t) for m in range(MT)
                        )
                    else:
                        fillers = deque(
                            lambda m=m, n=n, t=0: op_unit(m, n, 1, t)
                            for m in range(MT // 2) for n in range(N4)
                        )
                    place = PLACE[half]
                    frot = {"i": 0}

                    def fill(region, m):
                        if m in place[region] and fillers:
                            tags = FTAGS[half][region]
                            frot["i"] += 1
                            fillers.popleft()(t=tags[frot["i"] % len(tags)])

                    # R1a: scores h0
                    sc_rot["set"] = [0, 1, 2, 3, 4, 5] if half == 0 else [0, 1, 2, 3]
                    for m in range(MT):
                        fill("R1a", m)
                        sc_step(0, half, m)
                    # R1b: scores h1
                    for m in range(MT):
                        sc_step(1, half, m)
                        fill("R1b", m)
                    # R2: O h0 + scores h2
                    sc_rot["set"] = [0, 1]
                    for m in range(MT):
                        fill("R2", m)
                        o_step(0, m)
                        sc_step(2, half, m)
                    normalize(0, half)
                    # R3: O h1 + scores h3
                    for m in range(MT):
                        fill("R3", m)
                        o_step(1, m)
                        sc_step(3, half, m)
                    normalize(1, half)
                    # R4: O h2
                    for m in range(MT):
                        fill("R4", m)
                        o_step(2, m)
                    normalize(2, half)
                    # R5: O h3
                    for m in range(MT):
                        fill("R5", m)
                        o_step(3, m)
                    normalize(3, half)

                # leftover output projection (second Sq half rows); all psum
                # banks are free here, so rotate psf across them to overlap
                # accumulation with drains.  The first four units stage their
                # kh=0..2 accumulations before any kh=3 matmul so the final
                # h3 normalize chain hides behind real PE work.
                pro = []
                for u in range(4):
                    psf = ps(u, name="ps_f")
                    for kh in range(H - 1):
                        nc.tensor.matmul(
                            psf,
                            lhsT=oT[:, kh, (8 + u) * P:(9 + u) * P],
                            rhs=wo_sb[:, kh, 0:512],
                            start=(kh == 0),
                            stop=False,
                        )
                    pro.append(psf)
                for u in range(4):
                    nc.tensor.matmul(
                        pro[u],
                        lhsT=oT[:, 3, (8 + u) * P:(9 + u) * P],
                        rhs=wo_sb[:, 3, 0:512],
                        start=False,
                        stop=True,
                    )
                    ob = opool.tile([P, 512], bf16, tag="ob")
                    if u % 2 == 0:
                        nc.scalar.copy(ob, pro[u])
                    else:
                        nc.vector.tensor_copy(ob, pro[u])
                    dma.dma_start(out[(8 + u) * P:(9 + u) * P, 0:512], ob)
                for m in range(MT // 2, MT):
                    for n in range(N4):
                        if n == 0 and m < 12:
                            continue  # covered by the staged prologue
                        op_unit(m, n, (m + n) % 2, tag=(m * N4 + n) % 8,
                                split=(m == MT - 1 and n >= 2))

            stk.close()

    nc.compile()
    return nc


def _get_nc():
    if "nc" not in _CACHE:
        _CACHE["nc"] = _build_bass()
    return _CACHE["nc"]


def _prep_inputs(q, k, v, Wq, Wk, Wv, Wo):
    """Host-side sharding: per-core transposed bf16 slices."""
    scale = float(DH) ** -0.5
    q = np.asarray(q, np.float32)
    k = np.asarray(k, np.float32)
    v = np.asarray(v, np.float32)
    Wq = np.asarray(Wq, np.float32)
    Wk = np.asarray(Wk, np.float32)
    Wv = np.asarray(Wv, np.float32)
    Wo = np.asarray(Wo, np.float32)
    in_maps = []
    xT = {}
    for b in range(B):
        xT[b] = (
            q[b].T.astype(BF16),
            k[b].T.astype(BF16),
            v[b].T.astype(BF16),
        )
    for c in range(8):
        b, hg = divmod(c, 4)
        hs = hg * HS
        xqT, xkT, xvT = xT[b]
        in_maps.append(
            {
                "xq": xqT,
                "xk": xkT,
                "xv": xvT,
                "wq": np.ascontiguousarray((Wq[hs:hs + HS, :] * scale).T).astype(BF16),
                "wk": np.ascontiguousarray(Wk[hs:hs + HS, :].T).astype(BF16),
                "wv": np.ascontiguousarray(Wv[hs:hs + HS, :].T).astype(BF16),
                "wo": np.ascontiguousarray(Wo[:, hs:hs + HS].T).astype(BF16),
            }
        )
    return in_maps


def run_spmd(q, k, v, Wq, Wk, Wv, Wo, trace=False):
    from concourse.bass_utils import run_bass_kernel_spmd

    nc = _get_nc()
    in_maps = _prep_inputs(q, k, v, Wq, Wk, Wv, Wo)
    res = run_bass_kernel_spmd(nc, in_maps, list(range(8)), trace=trace)
    out = np.zeros((B, S, D), np.float32)
    for c in range(8):
        out[c // 4] += np.asarray(res.results[c]["out"], np.float32)
    return out, res


def kernel(q, k, v, mask, Wq, Wk, Wv, Wo):
    out, _ = run_spmd(q, k, v, Wq, Wk, Wv, Wo, trace=False)
    return out
